# revision 3
# baseline (speedup 1.0000x reference)
"""Edge-decoder (GNN link prediction) kernel for 8 Trainium2 NeuronCores.

Computes logits[e] = sum_d x[src[e], d] * x[tar[e], d] for 640K edges
(pos then neg), node table x [100000, 128] f32.

Strategy: edges sharded contiguously across 8 cores (80000/core), x
replicated in bf16 (host-converted; rel-err budget 2e-2 gives plenty of
room). Per core, edges are sorted by src and processed in 5 supergroups
of 16000. All row fetches use the Pool-engine custom bulk gather
(InstDMAGatherAnt via gpsimd.dma_gather, mlp ucode library), which
gathers thousands of 256B rows per instruction - measured ~500 GB/s/core
for random row streams when spread over the 4 SWDGE queues (vs ~30 GB/s
for the one-row-per-partition indirect DMA the previous kernel used).

dma_gather takes int16 indices, so a gather can only address a 32768-row
window:
  - src side: edges sorted by src mean a supergroup spans ~20K rows; the
    gather base is a runtime register (values_load of a host-computed
    per-supergroup base, clamped to N-32768 so idx16 always fits).
  - tar side: slots within a supergroup are grouped by tar range
    (4 static 32768-row windows; idx16 = tar - g*32768). Group slot
    capacities are padded to 128 and the index tails filled with -1
    (ignored; real count passed via num_idxs_reg from values_load).
Slots are shuffled within each group: random row order spreads reads
across HBM channels (measured 2.3x faster than sorted order).

Gathered src/tar tiles land as [128, blocks, 128] bf16 (row i of a
gather -> partition i%128, block i//128). DVE multiplies in place and
reduces over the feature dim to one f32 logit per slot; slot->edge
mapping is undone on the host.
"""

import numpy as np

N_NODES = 100000
D = 128
E_TOTAL = 640000
N_CORES = 8
P = 128
E_CORE = E_TOTAL // N_CORES  # 80000
NSG = 5
ESG = E_CORE // NSG  # 16000
RNG_ROWS = 32768  # int16-addressable window
NG = 4  # tar range groups: [0,32768), ..., [98304,100000)
CAPS = [5632, 5632, 5632, 512]  # slot capacity per tar group (x128 blocks)
CAP_BLKS = [c // P for c in CAPS]  # 44,44,44,4
SLOTS_SG = sum(CAPS)  # 17408
BLK_SG = SLOTS_SG // P  # 136
GRP_OFF = np.cumsum([0] + CAPS).tolist()  # slot offset of each group
IDX_COLS_SG = 2 * SLOTS_SG // 16  # int16 idx columns per supergroup: 2176
BASE_MAX = N_NODES - RNG_ROWS  # 67232

_cached = {}


def build(reps=1):
    import concourse.bacc as bacc
    import concourse.bass as bass
    import concourse.mybir as mybir
    from concourse.bass import ds
    from concourse.library_config import mlp

    nc = bacc.Bacc(
        "TRN2",
        target_bir_lowering=False,
        debug=False,
        num_devices=N_CORES,
        num_swdge_queues=4,
    )
    x = nc.dram_tensor("x", [N_NODES, D], mybir.dt.bfloat16, kind="ExternalInput")
    idx = nc.dram_tensor(
        "idx", [P, NSG * IDX_COLS_SG], mybir.dt.int16, kind="ExternalInput"
    )
    meta = nc.dram_tensor("meta", [1, 32], mybir.dt.int32, kind="ExternalInput")
    logits = nc.dram_tensor(
        "logits", [NSG, P, BLK_SG], mybir.dt.float32, kind="ExternalOutput"
    )

    with (
        nc.Block() as block,
        nc.sbuf_tensor("idx_sb", [P, NSG * IDX_COLS_SG], mybir.dt.int16) as idx_sb,
        nc.sbuf_tensor("meta_sb", [1, 32], mybir.dt.int32) as meta_sb,
        nc.sbuf_tensor("S", [P, 2, BLK_SG, D], mybir.dt.bfloat16) as S,
        nc.sbuf_tensor("T", [P, 2, BLK_SG, D], mybir.dt.bfloat16) as T,
        nc.sbuf_tensor("lg", [P, 2, BLK_SG], mybir.dt.float32) as lg,
        nc.semaphore("io") as io,
        nc.semaphore("g") as g,
        nc.semaphore("dv") as dv,
        nc.semaphore("st") as st,
    ):
        n_gs = reps * NSG

        @block.sync
        def _(sync):
            sync.dma_start(idx_sb[:], idx[:]).then_inc(io, 16)
            sync.dma_start(meta_sb[:], meta[:]).then_inc(io, 16)

        @block.gpsimd
        def _(gpsimd: bass.BassGpSimd):
            gpsimd.load_library(mlp)
            gpsimd.wait_ge(io, 32)
            base_regs = [
                nc.values_load(
                    meta_sb[0:1, s : s + 1],
                    engines=(mybir.EngineType.Pool,),
                    min_val=0,
                    max_val=BASE_MAX,
                    skip_runtime_bounds_check=True,
                )
                for s in range(NSG)
            ]
            cnt_regs = [
                [
                    nc.values_load(
                        meta_sb[0:1, 8 + s * NG + gi : 8 + s * NG + gi + 1],
                        engines=(mybir.EngineType.Pool,),
                        min_val=0,
                        max_val=CAPS[gi],
                        skip_runtime_bounds_check=True,
                    )
                    for gi in range(NG)
                ]
                for s in range(NSG)
            ]
            q = 0
            for gs in range(n_gs):
                s, b = gs % NSG, gs % 2
                if gs >= 2:
                    # reduce of gs-2 done -> S[b]/T[b] free
                    gpsimd.wait_ge(dv, gs - 1)
                col0 = s * IDX_COLS_SG
                for gi in range(NG):
                    cb0, cb1 = GRP_OFF[gi] // P, GRP_OFF[gi + 1] // P
                    ccols = CAPS[gi] // 16
                    rows = min(RNG_ROWS, N_NODES - gi * RNG_ROWS)
                    gpsimd.dma_gather(
                        S[:, b, cb0:cb1, :],
                        x.ap()[ds(base_regs[s], RNG_ROWS), :],
                        idx_sb[:, col0 : col0 + ccols],
                        CAPS[gi],
                        cnt_regs[s][gi],
                        D,
                        single_packet=False,
                        queue_num=q % 4,
                    ).then_inc(g, 16)
                    q += 1
                    gpsimd.dma_gather(
                        T[:, b, cb0:cb1, :],
                        x.ap()[gi * RNG_ROWS : gi * RNG_ROWS + rows, :],
                        idx_sb[:, col0 + ccols : col0 + 2 * ccols],
                        CAPS[gi],
                        cnt_regs[s][gi],
                        D,
                        single_packet=False,
                        queue_num=q % 4,
                    ).then_inc(g, 16)
                    q += 1
                    col0 += 2 * ccols
            gpsimd.wait_ge(g, 16 * 8 * n_gs)

        @block.vector
        def _(vector):
            import concourse.mybir as mybir_

            for gs in range(n_gs):
                b = gs % 2
                vector.wait_ge(g, 16 * 8 * (gs + 1))
                vector.tensor_tensor(
                    out=S[:, b],
                    in0=S[:, b],
                    in1=T[:, b],
                    op=mybir_.AluOpType.mult,
                )
                if gs >= 2:
                    # store of gs-2 done -> lg[b] free
                    vector.wait_ge(st, 16 * (gs - 1))
                vector.tensor_reduce(
                    out=lg[:, b],
                    in_=S[:, b],
                    axis=mybir_.AxisListType.X,
                    op=mybir_.AluOpType.add,
                ).then_inc(dv, 1)

        @block.scalar
        def _(scalar):
            for gs in range(n_gs):
                s, b = gs % NSG, gs % 2
                scalar.wait_ge(dv, gs + 1)
                scalar.dma_start(logits[s], lg[:, b]).then_inc(st, 16)
            scalar.wait_ge(st, 16 * n_gs)

    nc.compile()
    return nc


def _get_nc():
    if "nc" not in _cached:
        _cached["nc"] = build()
    return _cached["nc"]


def host_prepare(x, src, tar):
    """Per-core packing. Returns (in_maps, unpack) where unpack maps the
    kernel's logits outputs back to edge order."""
    import ml_dtypes

    xb = np.asarray(x, np.float32).astype(ml_dtypes.bfloat16)
    rng = np.random.default_rng(12345)
    in_maps, gathers = [], []
    for c in range(N_CORES):
        s_all = src[c * E_CORE : (c + 1) * E_CORE].astype(np.int64)
        t_all = tar[c * E_CORE : (c + 1) * E_CORE].astype(np.int64)
        perm = np.argsort(s_all, kind="stable")
        s_srt, t_srt = s_all[perm], t_all[perm]

        idx_blob = np.empty((16, 0), np.int16)
        bases = np.zeros(8, np.int32)
        counts = np.zeros(24, np.int32)
        slot_of = np.empty(E_CORE, np.int64)  # sorted-edge i -> sg*SLOTS_SG+slot
        for s in range(NSG):
            e0 = s * ESG
            ss, ts = s_srt[e0 : e0 + ESG], t_srt[e0 : e0 + ESG]
            base = min(int(ss[0]), BASE_MAX)
            assert int(ss[-1]) - base <= 32767, (c, s, ss[0], ss[-1])
            bases[s] = base
            cols = []
            gof = ts >> 15
            for gi in range(NG):
                in_g = np.where(gof == gi)[0]
                cnt = len(in_g)
                assert 0 < cnt <= CAPS[gi], (c, s, gi, cnt)
                counts[s * NG + gi] = cnt
                in_g = in_g[rng.permutation(cnt)]
                slot_of[e0 + in_g] = s * SLOTS_SG + GRP_OFF[gi] + np.arange(cnt)
                sl = np.full(CAPS[gi], -1, np.int16)
                sl[:cnt] = (ss[in_g] - base).astype(np.int16)
                tl = np.full(CAPS[gi], -1, np.int16)
                tl[:cnt] = (ts[in_g] - (gi << 15)).astype(np.int16)
                cols.append(sl.reshape(-1, 16).T)
                cols.append(tl.reshape(-1, 16).T)
            idx_blob = np.concatenate([idx_blob] + cols, axis=1)
        meta = np.zeros((1, 32), np.int32)
        meta[0, :8] = bases
        meta[0, 8:] = counts
        in_maps.append(
            {
                "x": xb,
                "idx": np.ascontiguousarray(np.tile(idx_blob, (8, 1))),
                "meta": meta,
            }
        )
        gathers.append((perm, slot_of))

    def unpack(results):
        out = np.empty((E_TOTAL, 1), np.float32)
        for c in range(N_CORES):
            perm, slot_of = gathers[c]
            lgv = results[c]["logits"]  # [NSG, P, BLK_SG]
            flat = lgv.transpose(0, 2, 1).reshape(-1)  # [sg*SLOTS_SG + blk*128+p]
            # slot s of sg -> flat index sg*SLOTS_SG + (s//P)*P... careful:
            # value for slot i lives at [sg, i%128, i//128] ->
            # transpose(0,2,1) gives [sg, i//128, i%128] -> flat idx
            # sg*SLOTS_SG + (i//128)*128 + (i%128) = sg*SLOTS_SG + i.
            srt = flat[slot_of]
            oc = np.empty(E_CORE, np.float32)
            oc[perm] = srt
            out[c * E_CORE : (c + 1) * E_CORE, 0] = oc
        return out

    return in_maps, unpack


def kernel(x, pos_edge_index, neg_edge_index):
    from concourse.bass_utils import run_bass_kernel_spmd

    src = np.concatenate(
        [np.asarray(pos_edge_index[0]), np.asarray(neg_edge_index[0])]
    ).astype(np.int32)
    tar = np.concatenate(
        [np.asarray(pos_edge_index[1]), np.asarray(neg_edge_index[1])]
    ).astype(np.int32)

    in_maps, unpack = host_prepare(x, src, tar)
    nc = _get_nc()
    res = run_bass_kernel_spmd(nc, in_maps, core_ids=list(range(N_CORES)))
    return unpack(res.results)
